# revision 44
# baseline (speedup 1.0000x reference)
"""Trainium2 Bass kernel for nn_Attention_36361193128703 (self-contained).

Entry point: kernel(**inputs) -> np.ndarray
  inputs: x (2,2048,1024) f32, w_in (3072,1024) f32,
          kernel_offsets/amplitudes/sharpness (16,16) f32
  returns: (2, 2048, 1024) f32 attention output (matches reference).

Distribution: 8 NeuronCores = data-parallel over batch (2) x tensor-parallel
over heads (4 head-groups of 4). Each core runs an identical single-core Bass
program on its shard; outputs are concatenated on the host. No collectives.

Core pipeline (per core: 4 heads = 2 head-pairs, L=2048):
  - TISA scores (reversed) -> g = exp(bias) staged in DRAM; 128 shifted rows
    per head loaded with positive-stride DMA (Toeplitz expansion); the reversal
    moves to a stride -1 free-dim read in the bias multiply.
  - Projections in bf16 (fp8 tested: weight-quantization noise amplifies to
    ~4% output error, over the 2e-2 tolerance).
  - S = k^T q for a head-pair: two K=64 matmuls packed concurrently into PE
    row-groups 0-63 / 64-127, one [128, 2, 512] PSUM pair tile.
  - P = exp(S/8) * g: one ACT exp (FD=1024) + one DVE multiply (2x mode).
  - O = V^T P accumulated in PSUM with a ones-column denominator row.
  - Epilogue (no PE, no ACT work): DVE evicts the accumulator to bf16, the
    DMA xbar transposes [64,512] -> [128,4,64] query-major, DVE reciprocal +
    broadcast-multiply normalizes, DMA out.
  ACT runs the 128 exps back-to-back (~1us each) and is the pacing engine;
  PE work (S + AV + projections) is scheduled to stay just under that pace.
"""
from collections import deque
from contextlib import ExitStack

import numpy as np

import concourse.bass as bass
import concourse.mybir as mybir
import concourse.tile as tile
from concourse import bacc
from concourse.bass import AP

F32 = mybir.dt.float32
BF16 = mybir.dt.bfloat16
U32 = mybir.dt.uint32

L = 2048
DM = 1024
HL = 4            # local heads
HD = 64
GW = 4096         # g window width per head (max needed index 4094)
IC = 512          # i-chunk (query) width per unit
NCH = L // IC     # 4 chunks
JT = 128          # j-tile (key) height
NJT = L // JT     # 16
NDC = DM // 128   # 8 d-chunks
AVLAG = 2         # units the AV stream trails S/exp (keeps PE FIFO unblocked
                  # across chunk boundaries while the epilogue drains PSUM)

# 1/sqrt(HD) is folded into the exp scale rather than the q weights.
QK_SCALE = 0.125
ONES_VAL = 1.0
# bf16 1.0 pair packed into a uint32 for the 2x-mode g-window memset
ONES_BF16x2 = 0x3F803F80


def build_kernel() -> bacc.Bacc:
    nc = bacc.Bacc("TRN2", target_bir_lowering=False, debug=False, num_devices=8)

    # x and w arrive as separate contiguous tensors so every input DMA is a
    # full-tensor (or full-chunk) copy with maximal run length: big packets,
    # cheap descriptor generation, and completion semaphores that gate only
    # the consumers that actually need that piece.
    xA_d = nc.dram_tensor("xA", [128, NDC, 512], BF16, kind="ExternalInput")
    xB_d = [nc.dram_tensor(f"xB{i}", [128, NDC, 512], BF16,
                           kind="ExternalInput") for i in range(3)]
    wk0_d = nc.dram_tensor("wk0", [128, NDC, 128], BF16, kind="ExternalInput")
    wk1_d = nc.dram_tensor("wk1", [128, NDC, 128], BF16, kind="ExternalInput")
    wq0_d = nc.dram_tensor("wq0", [128, NDC, 128], BF16, kind="ExternalInput")
    wq1_d = nc.dram_tensor("wq1", [128, NDC, 128], BF16, kind="ExternalInput")
    wv_d = nc.dram_tensor("wv", [128, NDC, 256], BF16, kind="ExternalInput")
    tisa_d = nc.dram_tensor("tisa", [64, 6], F32, kind="ExternalInput")
    ramp_d = nc.dram_tensor("ramp", [64, 1024], F32, kind="ExternalInput")
    out_d = nc.dram_tensor("out", [L, 256], F32, kind="ExternalOutput")

    dma_engines = [nc.sync, nc.gpsimd]

    def dma(i, out, in_):
        dma_engines[i % len(dma_engines)].dma_start(out, in_)

    with tile.TileContext(nc) as tc, ExitStack() as ctx:
        const_pool = ctx.enter_context(tc.tile_pool(name="const", bufs=1))

        # PSUM budget (8 banks): kq-proj 1, vproj 1, S double-buffer 4, AV 2.
        kq_psum = ctx.enter_context(tc.tile_pool(name="kqps", bufs=1, space="PSUM"))
        vp_psum = ctx.enter_context(tc.tile_pool(name="vpps", bufs=1, space="PSUM"))
        s_psum = ctx.enter_context(tc.tile_pool(name="sps", bufs=2, space="PSUM"))
        o_psum = ctx.enter_context(tc.tile_pool(name="ops", bufs=1, space="PSUM"))

        gdram_pool = ctx.enter_context(tc.tile_pool(name="gdram", bufs=1, space="DRAM"))
        g_band = gdram_pool.tile([HL * 1024], BF16)
        gbase = g_band[:]

        # ---------------- input DMAs -------------------------------------------
        # tisa first: it is tiny and gates the whole phase-0 chain.
        tp = ctx.enter_context(tc.tile_pool(name="tisa_tmp", bufs=1))
        tisa_sb = tp.tile([64, 6], F32)
        nc.sync.dma_start(tisa_sb[:, :], tisa_d[:, :])
        rsb = tp.tile([64, 1024], F32, tag="ramp")
        nc.gpsimd.dma_start(rsb[:, :], ramp_d[:, :])
        xpool = ctx.enter_context(tc.tile_pool(name="xT", bufs=1))
        wpool = ctx.enter_context(tc.tile_pool(name="w", bufs=1))
        xT_sb = xpool.tile([128, NDC, L], BF16, name="xT", tag="xT")
        wk0_sb = wpool.tile([128, NDC, 128], BF16, name="wk0", tag="wk0")
        wk1_sb = wpool.tile([128, NDC, 128], BF16, name="wk1", tag="wk1")
        wq0_sb = wpool.tile([128, NDC, 128], BF16, name="wq0", tag="wq0")
        wq1_sb = wpool.tile([128, NDC, 128], BF16, name="wq1", tag="wq1")
        wv_sb = wpool.tile([128, NDC, 256], BF16, name="wv", tag="wv")
        w_ec = [wk0_sb, wk1_sb, wq0_sb, wq1_sb]
        # Queue plan (each issue queue drains in order, so position = priority):
        #   gpsimd: ramp, x tokens 0-511 per d-chunk, then the x tail in
        #           512-token waves matching when the later kq tiles need them.
        #   scalar: the three weight groups the pre-phase consumes.
        #   sync:   tisa now; g-store/windows and the late weights follow in
        #           emission order so the g chain is never stuck behind bulk.
        for dc in range(NDC):
            nc.gpsimd.dma_start(xT_sb[:, dc:dc + 1, 0:512],
                                xA_d[:, dc:dc + 1, :])
        nc.scalar.dma_start(wk0_sb[:, :, :], wk0_d[:, :, :])
        nc.scalar.dma_start(wq0_sb[:, :, :], wq0_d[:, :, :])
        nc.sync.dma_start(wv_sb[:, :, :], wv_d[:, :, :])
        # x tail waves as d-chunk pairs: half the descriptor-generation slots
        # on the gpsimd queue, so the last wave is issued (and lands) before
        # the first-chunk kq tiles that consume it come due.
        for i in range(3):
            t0 = 512 + i * 512
            for dc in range(0, NDC, 2):
                nc.gpsimd.dma_start(xT_sb[:, dc:dc + 2, t0:t0 + 512],
                                    xB_d[i][:, dc:dc + 2, :])

        # ---------------- Phase 0: TISA scores (reversed) -> g_pad ---------------
        # ev[:, m] = (L-1) - m so g_pad holds rev[m] = score[2L-2 - m]; the
        # Toeplitz flip then needs only positive DMA strides.
        # Band-windowed TISA: the Gaussian mixture has reach << L for these
        # params (host asserts max|off| + width < 255), so scores are computed
        # only on rev-band [1536, 2560); everywhere else exp(bias) == 1.0 in
        # bf16 and the g tiles are simply memset to 1.
        # tisa col0 = -off, col1 = -|sharp| (negated on the host), so the ACT
        # affine stage does the whole pointwise chain in two table-shared ops:
        # u = (ramp + (-off))^2, then evb = exp((-|sh|) * u).
        ev = tp.tile([64, 1024], F32, tag="scr")
        evb = tp.tile([64, 1024], BF16, tag="scrb")
        ampb = tp.tile([64, 4], BF16)
        nc.vector.tensor_copy(ampb[:, :], tisa_sb[:, 2:6])
        # padding: keeps the hot pools (kq/V/grep/p/es) at the SBUF offsets of
        # the best-measured layout (downstream tile placement is perf-sensitive,
        # likely via SBUF bank conflicts between concurrent engine accesses).
        # The g-window pool moved up here (its 1.0-fill must lead the DVE
        # queue), so padt shrank by the 32KB it occupies and pad2 below fills
        # the slot it vacated.
        padt = tp.tile([128, 768], F32, tag="pad", name="padt")
        nc.scalar.activation(ev[:, :], rsb[:, :],
                             mybir.ActivationFunctionType.Square,
                             bias=tisa_sb[:, 0:1])
        nc.scalar.activation(evb[:, :], ev[:, :],
                             mybir.ActivationFunctionType.Exp,
                             scale=tisa_sb[:, 1:2])

        # g windows: 1.0-fill as packed uint32 writes (half the elements, 2x
        # DVE mode), emitted before any x-gated DVE work so they lead the
        # vector queue and finish during the input-DMA wait.
        grep_pool = ctx.enter_context(tc.tile_pool(name="grep", bufs=1))
        grepp = [grep_pool.tile([128, 2, GW], BF16, name=f"grep{pp}",
                                tag=f"grep{pp}") for pp in range(2)]
        # grep[0]'s fill leads the DVE queue; grep[1]'s (not needed until unit
        # 64) is deferred into the unit stream so it never delays unit 0.
        nc.vector._memset_packed(grepp[0][:, :, :].bitcast(U32), ONES_BF16x2)

        def emit_mg1(quarter):
            # one [128, 1, 2048-word] slice at a time (~0.9us each) so the
            # fill never blocks a unit's multiply in the DVE FIFO.
            h, half = quarter // 2, quarter % 2
            gu = grepp[1][:, h:h + 1, half * 2048:(half + 1) * 2048]
            nc.vector._memset_packed(gu.bitcast(U32), ONES_BF16x2)

        def emit_phase0_mms():
            gch = tp.tile([HL, 1024], BF16, tag="gch")
            for mc in range(2):
                ps = o_psum.tile([65, 512], F32, tag=f"O{mc}", name="ph0")
                nc.tensor.matmul(ps[0:HL, :], ampb[:, :],
                                 evb[:, mc * 512:(mc + 1) * 512],
                                 start=True, stop=True)
                nc.scalar.activation(gch[:, mc * 512:(mc + 1) * 512],
                                     ps[0:HL, :],
                                     mybir.ActivationFunctionType.Exp)
            dst = AP(gbase.tensor, gbase.offset, [[1024, HL], [1, 1024]])
            nc.sync.dma_start(dst, gch[:, :])

        # ---------------- projections ------------------------------------------
        kq_pool = ctx.enter_context(tc.tile_pool(name="kq", bufs=1))
        v_pool = ctx.enter_context(tc.tile_pool(name="V", bufs=1))
        kq_sb = [[kq_pool.tile([128, 512], BF16, name=f"kq{i}_{t}",
                          tag=f"kq{i}_{t}") for t in range(4)]
                 for i in range(4)]
        # V tiles are pre-created and their ones-columns written up front on
        # the (otherwise idle) DVE so no projection waits on a memset later.
        v_sb = [v_pool.tile([128, HL, 65], BF16, name=f"v{tt}", tag=f"v{tt}")
                for tt in range(NJT)]
        for tt in range(NJT):
            nc.vector.memset(v_sb[tt][:, :, 64:65], ONES_VAL)

        kq_pend = {}

        def emit_kq_quarter(ec, tcn, quarter):
            """Two of the eight K-accumulation matmuls for one kq tile."""
            if quarter == 0:
                ps = kq_psum.tile([128, 512], F32, tag="kqa",
                                  name=f"kqp{ec}{tcn}")
                kq_pend[(ec, tcn)] = ps
            else:
                ps = kq_pend[(ec, tcn)]
            for k in range(2):
                kidx = quarter * 2 + k
                dc = (tcn * 2 + kidx) % NDC
                nc.tensor.matmul(ps[:, :],
                                 w_ec[ec][:, dc:dc + 1, :],
                                 xT_sb[:, dc:dc + 1, tcn * 512:(tcn + 1) * 512],
                                 start=(kidx == 0), stop=(kidx == NDC - 1))
            if quarter == 3:
                del kq_pend[(ec, tcn)]
                nc.vector.tensor_copy(kq_sb[ec][tcn][:, :], ps[:, :])

        def emit_kq_tcn(ec, tcn):
            for q in range(4):
                emit_kq_quarter(ec, tcn, q)

        def emit_vproj_tt(tt):
            ps = vp_psum.tile([128, 512], F32, tag="vpa", name="ps")
            for dc in range(NDC):
                nc.tensor.matmul(ps[:, 0:256],
                                 xT_sb[:, dc:dc + 1, tt * 128:(tt + 1) * 128],
                                 wv_sb[:, dc:dc + 1, :],
                                 start=(dc == 0), stop=(dc == NDC - 1))
            psa = ps[:, 0:256]
            ps3 = AP(psa.tensor, psa.offset, [psa.ap[0], [64, HL], [1, 64]])
            nc.vector.tensor_copy(v_sb[tt][:, :, 0:64], ps3)

        # Pre-phase: phase-0 matmuls first (they gate the g-window chain and
        # need only the tiny tisa/ramp inputs), then the four projection tiles
        # the first units consume -- kq(0,0), kq(2,0), V0, V1 -- accumulated
        # with their d-chunk matmuls interleaved so each runs as its x chunk
        # arrives.  kq00/kq20 borrow the two S-ring PSUM banks (idle until the
        # first units), leaving the kq/vproj banks for V0/V1.
        psA = s_psum.tile([128, 2, IC], F32, tag="S", name="preA")
        psB = s_psum.tile([128, 2, IC], F32, tag="S", name="preB")
        pv0 = kq_psum.tile([128, 512], F32, tag="kqa", name="pre_v0")
        pv1 = vp_psum.tile([128, 512], F32, tag="vpa", name="pre_v1")

        def pre_kq(dc):
            st, sp = dc == 0, dc == NDC - 1
            nc.tensor.matmul(psA[:, 0:1, :], wk0_sb[:, dc:dc + 1, :],
                             xT_sb[:, dc:dc + 1, 0:512], start=st, stop=sp)
            nc.tensor.matmul(psB[:, 0:1, :], wq0_sb[:, dc:dc + 1, :],
                             xT_sb[:, dc:dc + 1, 0:512], start=st, stop=sp)

        def pre_v(dc):
            st, sp = dc == 0, dc == NDC - 1
            nc.tensor.matmul(pv0[:, 0:256], xT_sb[:, dc:dc + 1, 0:128],
                             wv_sb[:, dc:dc + 1, :], start=st, stop=sp)
            nc.tensor.matmul(pv1[:, 0:256], xT_sb[:, dc:dc + 1, 128:256],
                             wv_sb[:, dc:dc + 1, :], start=st, stop=sp)

        # Phase-0 matmuls lead the Tensor queue (they gate the whole g-window
        # chain and need only evb); kq(0,0)/kq(2,0) follow, paced by the xA
        # arrivals, with the V0/V1 accumulations trailing a few d-chunks so
        # the kq tiles finish (and copy out) first.
        emit_phase0_mms()
        for dc in range(3):
            pre_kq(dc)
        for dc in range(3, NDC):
            pre_kq(dc)
            pre_v(dc - 3)
        nc.vector.tensor_copy(kq_sb[0][0][:, :], psA[:, 0, :])
        nc.vector.tensor_copy(kq_sb[2][0][:, :], psB[:, 0, :])
        for dc in range(NDC - 3, NDC):
            pre_v(dc)
        for tt, pv in ((0, pv0), (1, pv1)):
            psa = pv[:, 0:256]
            ps3 = AP(psa.tensor, psa.offset, [psa.ap[0], [64, HL], [1, 64]])
            nc.vector.tensor_copy(v_sb[tt][:, :, 0:64], ps3)

        # pad2 fills the SBUF slot the g-window pool vacated so the pools
        # below keep their measured offsets.
        pad2_pool = ctx.enter_context(tc.tile_pool(name="pad2", bufs=1))
        pad2_pool.tile([128, 8192], F32, tag="pad2", name="pad2")

        # g-window band loads (overwrite the middle of the 1.0-fill).  The
        # pair-0 windows go first; the late weights slot in before the pair-1
        # windows, whose issue blocks on the deferred grep[1] fill.
        def emit_window(hi):
            src = AP(gbase.tensor, gbase.offset + hi * 1024 + 129,
                     [[1, 128], [1, 640]])
            nc.sync.dma_start(grepp[hi // 2][:, hi % 2:hi % 2 + 1, 1665:2305],
                              src)

        emit_window(0)
        emit_window(1)
        nc.sync.dma_start(wk1_sb[:, :, :], wk1_d[:, :, :])
        nc.sync.dma_start(wq1_sb[:, :, :], wq1_d[:, :, :])
        # windows 2/3 are emitted after the deferred grep[1] fill (unit 7) so
        # the fill cannot overwrite them.

        # ---------------- Phase 3: attention units -------------------------------
        p_pool = ctx.enter_context(tc.tile_pool(name="p", bufs=6))
        e_pool = ctx.enter_context(tc.tile_pool(name="es", bufs=6))
        o_pool = ctx.enter_context(tc.tile_pool(name="o", bufs=2))
        r_pool = ctx.enter_context(tc.tile_pool(name="r", bufs=2))
        out_pool = ctx.enter_context(tc.tile_pool(name="out", bufs=2))
        nt_pool = ctx.enter_context(tc.tile_pool(name="nt", bufs=2))



        def emit_S(pp, c, jt):
            """Row-packed head-pair S matmuls -> [128, 2, 512] PSUM tile."""
            ps = s_psum.tile([128, 2, IC], F32, tag="S", name=f"s{pp}_{c}_{jt}")
            kqt = kq_sb[pp][jt // 4]
            qqt = kq_sb[2 + pp][c]
            joff = (jt % 4) * JT
            for h in range(2):
                pb = h * 64
                nc.tensor.matmul(ps[:, h:h + 1, :],
                                 kqt[pb:pb + 64, joff:joff + JT],
                                 qqt[pb:pb + 64, :],
                                 start=True, stop=True)
            return ps

        def emit_expmult(pp, c, jt, ps, eng):
            es = e_pool.tile([128, 2, IC], BF16, tag="es", name=f"e{pp}_{c}_{jt}")
            nc.scalar.activation(es[:, :, :], ps[:, :, :],
                                 mybir.ActivationFunctionType.Exp,
                                 scale=QK_SCALE)
            u0 = (L - 1) - c * IC + jt * JT
            # Units whose whole relative-position span lies outside the loaded
            # TISA band read only the 1.0-fill: skip the multiply and feed the
            # exp result straight to AV (bit-exact).
            if not (1665 <= u0 <= 2304 + IC - 1):
                return es
            pt = p_pool.tile([128, 2, IC], BF16, tag="p", name=f"p{pp}_{c}_{jt}")
            g2 = grepp[pp][:, :, :]
            g_rev = AP(g2.tensor, g2.offset + u0, [g2.ap[0], [GW, 2], [-1, IC]])
            eng.tensor_mul(pt[:, :, :], es[:, :, :], g_rev)
            return pt

        psos = {}

        def emit_AV(pp, c, jt, pt):
            if jt == 0:
                psos[0] = o_psum.tile([65, IC], F32, tag="O0", name=f"o0_{pp}_{c}")
                psos[1] = o_psum.tile([65, IC], F32, tag="O1", name=f"o1_{pp}_{c}")
            for h in range(2):
                hi = 2 * pp + h
                nc.tensor.matmul(psos[h][:, :],
                                 v_sb[jt][:, hi:hi + 1, :],
                                 pt[:, h:h + 1, :],
                                 start=(jt == 0), stop=(jt == NJT - 1))
            if jt == NJT - 1:
                # staged epilogue: spread over the next units so no engine's
                # FIFO blocks on an in-flight transpose DMA.
                emit_epilogue_stage(pp, c, 0)
                ep_pend.extend([(pp, c, 1), (pp, c, 2), (pp, c, 3)])

        ep_pend = []
        ep_state = {}

        def emit_epilogue_stage(pp, c, stage):
            """Normalize + transpose one chunk's AV accumulators.

            stage 0 (with the jt=15 AV): head-0 evict (num rows 0-63 + den row
              64, one bf16 copy) and its transpose DMA; den rides along as
              column 64 of the transposed tile.
            stage 1 (+1 unit): head-1 evict + transpose DMA (frees PSUM).
            stage 2 (+2): head-0 reciprocal (FD=4 post-transpose, cheap) +
              broadcast-multiply + output DMA.
            stage 3 (+3): same for head 1.
            """
            i0 = c * IC

            def evict_and_transpose(h):
                # [80, 512] staging: the xbar needs a multiple-of-16 source
                # partition count; rows 65-79 are junk and land in unread
                # columns 65-79 of the transpose.
                osb = o_pool.tile([80, IC], BF16, tag="osb",
                                  name=f"ob{pp}_{c}_{h}")
                nc.gpsimd.memset(osb[64:80, :], 0.0)
                # last chunk: the exp stream is over, so ACT takes one evict
                # and the scalar HWDGE queue one transpose -- the final
                # two-head drain runs on parallel engines.
                last = (pp, c, h) == (1, NCH - 1, 1)
                if last:
                    nc.scalar.copy(osb[0:65, :], psos[h][:, :])
                else:
                    nc.vector.tensor_copy(osb[0:65, :], psos[h][:, :])
                ntl = nt_pool.tile([128, 4, 80], BF16, tag=f"nt{h}",
                                   name=f"nt{pp}_{c}_{h}")
                eng = nc.scalar if last else nc.sync
                eng.dma_start(ntl[:, :, :], osb[:, :], transpose=True)
                ep_state[(pp, c, h)] = ntl

            def mul_and_out(h):
                hi = 2 * pp + h
                ntl = ep_state.pop((pp, c, h))
                rctq = r_pool.tile([128, 4, 1], BF16, tag=f"rq{h}",
                                   name=f"rq{pp}_{c}_{h}")
                with nc.allow_low_precision(reason="bf16 softmax denom"):
                    nc.vector.reciprocal(rctq[:, :, :], ntl[:, :, 64:65])
                ot = out_pool.tile([128, 4, HD], F32, tag="ot", name="ot")
                nc.vector.tensor_mul(ot[:, :, :], ntl[:, :, 0:64],
                                     rctq[:, :, :].broadcast_to([128, 4, HD]))
                ob = out_d[:, :]
                dst = AP(ob.tensor, ob.offset + i0 * 256 + hi * HD,
                         [[256, 128], [128 * 256, 4], [1, HD]])
                dma(hi + c, dst, ot[:, :, :])

            if stage == 0:
                evict_and_transpose(0)
            elif stage == 1:
                evict_and_transpose(1)
            elif stage == 2:
                mul_and_out(0)
            else:
                mul_and_out(1)

        # late-projection schedule: unit index -> list of (ec, tcn, quarter).
        # Deadlines (unit whose S consumes the tile): kq(0,t) by unit 4t,
        # kq(2,c) by unit 16c, kq(1,t) by 64+4t, kq(3,c) by 64+16c.  One
        # quarter (2 matmuls) per unit, each block done a unit early so the
        # consuming S never queues behind its own producer.
        sched = {}
        _order = [(0, 1, 0), (0, 2, 4), (0, 3, 8), (2, 1, 12), (2, 2, 16),
                  (2, 3, 20), (1, 0, 24), (3, 0, 28), (1, 1, 32), (3, 1, 36),
                  (1, 2, 40), (3, 2, 44), (1, 3, 48), (3, 3, 52)]
        for ec, tcn, u0_ in _order:
            for q in range(4):
                sched.setdefault(u0_ + q, []).append((ec, tcn, q))

        units = [(pp, c, jt) for pp in range(2) for c in range(NCH)
                 for jt in range(NJT)]
        av_pend = deque()
        for idx, (pp, c, jt) in enumerate(units):
            ps = emit_S(pp, c, jt)
            if idx < NJT - 2:
                emit_vproj_tt(idx + 2)
            for item in sched.get(idx, ()):
                emit_kq_quarter(*item)
            if idx in (8, 10, 12, 14):
                emit_mg1((idx - 8) // 2)
            if idx == 15:
                emit_window(2)
                emit_window(3)
            pt = emit_expmult(pp, c, jt, ps, nc.vector)
            while ep_pend:
                emit_epilogue_stage(*ep_pend.pop(0))
            av_pend.append((pp, c, jt, pt))
            if len(av_pend) > AVLAG:
                emit_AV(*av_pend.popleft())
        while av_pend:
            emit_AV(*av_pend.popleft())
            while ep_pend:
                emit_epilogue_stage(*ep_pend.pop(0))
        while ep_pend:
            emit_epilogue_stage(*ep_pend.pop(0))

    nc.compile()
    return nc


def shard_inputs(inputs: dict) -> list[dict]:
    """Full inputs -> 8 per-core input maps (bf16/f32 prep for the device)."""
    import ml_dtypes

    x, w_in = inputs["x"], inputs["w_in"]
    off = inputs["kernel_offsets"]
    amp = inputs["kernel_amplitudes"]
    sh = inputs["kernel_sharpness"]
    D = DM
    in_maps = []
    for c in range(8):
        b, hg = c // 4, c % 4
        heads = list(range(4 * hg, 4 * hg + 4))
        # xT8[p, dc, t] = x[b, t, dc*128+p]
        xT8 = np.ascontiguousarray(
            x[b].T.reshape(NDC, 128, L).transpose(1, 0, 2)
        ).astype(ml_dtypes.bfloat16)
        xA8 = np.ascontiguousarray(xT8[:, :, 0:512])
        xB8 = {f"xB{i}": np.ascontiguousarray(
            xT8[:, :, 512 + i * 512:1024 + i * 512]) for i in range(3)}
        rows_k = np.concatenate([w_in[h * HD:(h + 1) * HD] for h in heads])
        rows_q = np.concatenate(
            [w_in[2 * D + h * HD:2 * D + (h + 1) * HD] for h in heads])
        rows_v = np.concatenate([w_in[D + h * HD:D + (h + 1) * HD] for h in heads])
        wfeat = np.concatenate([rows_k, rows_q, rows_v])
        # w8[p, dc, e] = wfeat[e, dc*128+p]
        w8 = np.ascontiguousarray(
            wfeat.T.reshape(NDC, 128, 768).transpose(1, 0, 2)
        ).astype(ml_dtypes.bfloat16)
        wparts = {n: np.ascontiguousarray(w8[:, :, a:b]) for n, a, b in
                  (("wk0", 0, 128), ("wk1", 128, 256), ("wq0", 256, 384),
                   ("wq1", 384, 512), ("wv", 512, 768))}
        tisa = np.zeros((64, 6), np.float32)
        tisa[:, 0] = -off[heads].reshape(-1)
        tisa[:, 1] = -np.abs(sh[heads].reshape(-1))
        for hi in range(4):
            tisa[hi * 16:(hi + 1) * 16, 2 + hi] = amp[heads[hi]]
        reach = np.abs(off).max() + np.sqrt(
            np.log(max(np.abs(amp).max(), 1e-3) / 1e-3)
            / max(np.abs(sh).min(), 1e-4))
        assert reach < 255.0, f"TISA reach {reach} exceeds band window"
        # rev-band m in [1536, 2560): rel = 2047 - m = 511 - j
        ramp = np.broadcast_to(
            (np.float32(511) - np.arange(1024, dtype=np.float32))[None, :],
            (64, 1024)).copy()
        in_maps.append({"xA": xA8, "tisa": tisa, "ramp": ramp,
                        **xB8, **wparts})
    return in_maps


def unshard_output(results: list[dict]) -> np.ndarray:
    out = np.zeros((2, L, DM), np.float32)
    for c in range(8):
        b, hg = c // 4, c % 4
        out[b, :, hg * 256:(hg + 1) * 256] = results[c]["out"]
    return out


_NC_CACHE = None


def kernel(**inputs) -> np.ndarray:
    global _NC_CACHE
    from concourse.bass_utils import run_bass_kernel_spmd

    if _NC_CACHE is None:
        _NC_CACHE = build_kernel()
    in_maps = shard_inputs({k: np.asarray(v) for k, v in inputs.items()})
    res = run_bass_kernel_spmd(_NC_CACHE, in_maps, core_ids=list(range(8)))
    return unshard_output(res.results)
